# revision 54
# baseline (speedup 1.0000x reference)
"""Multi-head self-attention (RoPE + diagonal mask) TRN2 Bass kernel, 8-core SPMD.

Sharding: core = batch*2 + head_half. Each core computes, for its batch and its
8 heads: QKV projection (fp16 matmuls, f32 PSUM), RoPE, transposed scores
S^T = K @ Q^T, exp (no max-subtraction - scores are bounded), diagonal mask as
post-exp zeroing, P^T @ V_aug where V is augmented with 64 ones-columns so the
PV matmul (cost depends only on N) yields the softmax denominators REPLICATED
across partitions 64..127 - normalization is then a reciprocal + multiply with
no partition broadcast. Output projection restricted to this core's 512 rows
of Wproj; t0+t1 partials are pair-accumulated in PSUM; partial outputs are
summed on the host (tensor-parallel reduce).

Schedule: one global chunk pipeline over 16 (head, q-half) segments x 16 key
chunks. Scores prefetch runs 2 chunks ahead and crosses segment boundaries;
QKV / RoPE / V / projection granules are placed per-slot from a static plan
that equalizes PE time per slot (~1.2us) so ScalarE's exp (~1.04us/chunk)
never stalls the PE. A warm-up matmul run bridges the input-DMA latency so
the PE p-state never drops back to half clock.

_build_nc(reps=N) emits the whole body (input DMAs included) N times into one
NEFF - used by test.py for single-dispatch steady-state timing.
"""
import sys

sys.path.insert(0, "/opt/trn_rl_repo")

import numpy as np

import concourse.mybir as mybir
import concourse.tile as tile
from concourse import bacc
from concourse.bass_utils import run_bass_kernel_spmd

FP16 = mybir.dt.float16
F32 = mybir.dt.float32

B = 4
S = 2048
DM = 1024
NH = 16
HD = 64
H_CORE = 8          # heads per core
N_CORES = 8
KT = DM // 128      # 8 k-tiles over the model dim
SC = S // 128       # 16 seq chunks of 128
SCALE = HD ** -0.5
N_WARM = 36         # p-state ramp matmuls (N=256) bridging input-DMA latency

SWAP_MASK = []
for _i in range(16):
    SWAP_MASK += [2 * _i + 1, 2 * _i]

_CACHE = {}


def _build_nc(reps=1):
    nc = bacc.Bacc("TRN2", target_bir_lowering=False, debug=False, num_devices=N_CORES)

    xT_d = nc.dram_tensor("xT", [DM, S], FP16, kind="ExternalInput").ap()
    wq_d = nc.dram_tensor("wq", [DM, 512], FP16, kind="ExternalInput").ap()
    wk_d = nc.dram_tensor("wk", [DM, 512], FP16, kind="ExternalInput").ap()
    wv_d = nc.dram_tensor("wv", [DM, 512], FP16, kind="ExternalInput").ap()
    wp_d = nc.dram_tensor("wp", [512, DM], FP16, kind="ExternalInput").ap()
    cos_d = nc.dram_tensor("cosb", [128, S], FP16, kind="ExternalInput").ap()
    sin_d = nc.dram_tensor("sinb", [128, S], FP16, kind="ExternalInput").ap()
    msk_d = nc.dram_tensor("dmask", [128, 128], FP16, kind="ExternalInput").ap()
    # out01 = (t0+t1) PSUM-paired partials; t2 and t3 stream separately (their
    # yn halves finish too late to pair without reading ahead of the write).
    out01_d = nc.dram_tensor("out01", [S, DM], FP16, kind="ExternalOutput").ap()
    out2_d = nc.dram_tensor("out2", [S, DM], FP16, kind="ExternalOutput").ap()
    out3_d = nc.dram_tensor("out3", [S, DM], FP16, kind="ExternalOutput").ap()
    Exp = mybir.ActivationFunctionType.Exp

    with tile.TileContext(nc) as tc:
        with (
            tc.tile_pool(name="consts", bufs=1) as consts,
            tc.tile_pool(name="phb", bufs=1) as phb,
            tc.tile_pool(name="rope", bufs=2) as ropep,
            tc.tile_pool(name="pt", bufs=6) as ptp,
            tc.tile_pool(name="rzb", bufs=2) as rzbp,
            tc.tile_pool(name="yab", bufs=2) as yap,
            tc.tile_pool(name="outsb", bufs=4) as outp,
            tc.tile_pool(name="sps", bufs=2, space="PSUM") as spsp,
            tc.tile_pool(name="pvps", bufs=1, space="PSUM") as pvpsp,
            tc.tile_pool(name="aux", bufs=2, space="PSUM") as auxp,
        ):
            # ---- persistent tiles ----
            cos_sb = consts.tile([128, S], FP16)
            sin_sb = consts.tile([128, S], FP16)
            msk_sb = consts.tile([128, 128], FP16)
            wp_sb = consts.tile([128, 4, DM], FP16)
            warm_sb = consts.tile([128, 512], FP16)

            kT = [consts.tile([128, S], FP16, name=f"kT{t}", tag=f"kT{t}") for t in range(4)]
            qT = [consts.tile([128, S], FP16, name=f"qT{t}", tag=f"qT{t}") for t in range(4)]
            yn = [consts.tile([128, S], FP16, name=f"yn{t}", tag=f"yn{t}") for t in range(4)]
            # V augmented with 64 ones-columns: PV output rows 64..127 = denominators
            v_sb = consts.tile([128, SC, H_CORE, 2 * HD], FP16)

            xT_sb = phb.tile([128, KT, S], FP16)
            wq_sb = phb.tile([128, KT, 512], FP16)
            wk_sb = phb.tile([128, KT, 512], FP16)
            wv_sb = phb.tile([128, KT, 512], FP16)

            # memsets on Pool so DVE is free for the first ropes; only the
            # columns the warm-up matmuls touch, so the PE starts ASAP
            nc.gpsimd.memset(warm_sb[:, 0:256], 0.0)
            for sc4 in range(0, SC, 4):
                nc.gpsimd.memset(v_sb[:, sc4:sc4 + 4, :, HD:2 * HD], 1.0)

            # ---- PE warm-up: ramp the p-state while input DMAs stream.
            # Sized so real matmuls start back-to-back with the warm-up at
            # full clock - a PE idle gap would reset the ramp to half speed.
            # The warm psum lives in the pv pool so aux stays free for the
            # prologue's QKV accumulations.
            warm_ps = pvpsp.tile([128, 1024], F32, tag="pv", name="warmps")

            def warm2(n):
                for _ in range(n):
                    nc.tensor.matmul(warm_ps[:, 0:256], warm_sb[:, 0:128],
                                     warm_sb[:, 0:256], start=True, stop=True)

            warm2(N_WARM)
            for rep in range(reps):
                _emit_body(nc, tc, rep, warm2 if rep == 0 else None, locals())

    nc.compile()
    return nc


def _emit_body(nc, tc, rep, warm2, env):
    """One full pass: input DMAs, QKV+RoPE, attention pipeline, projection.
    All tile names are suffixed per rep so reps=N builds N serialized passes
    in one NEFF (test.py uses that for single-dispatch steady-state timing).
    """
    R = f"r{rep}"
    Exp = mybir.ActivationFunctionType.Exp
    xT_d, wq_d, wk_d, wv_d, wp_d = (env[k] for k in
                                    ("xT_d", "wq_d", "wk_d", "wv_d", "wp_d"))
    cos_d, sin_d, msk_d = env["cos_d"], env["sin_d"], env["msk_d"]
    out01_d, out2_d, out3_d = env["out01_d"], env["out2_d"], env["out3_d"]
    cos_sb, sin_sb, msk_sb, wp_sb = (env[k] for k in
                                     ("cos_sb", "sin_sb", "msk_sb", "wp_sb"))
    kT, qT, yn, v_sb = env["kT"], env["qT"], env["yn"], env["v_sb"]
    xT_sb, wq_sb, wk_sb, wv_sb = (env[k] for k in
                                  ("xT_sb", "wq_sb", "wk_sb", "wv_sb"))
    ropep, ptp, rzbp, yap, outp = (env[k] for k in
                                   ("ropep", "ptp", "rzbp", "yap", "outp"))
    spsp, pvpsp, auxp = env["spsp"], env["pvpsp"], env["auxp"]

    # ---- input DMAs, in startup-criticality order (the model serializes
    # transfer time, so order ~= landing order). 256-col weight slices keep
    # the contiguous element >= 512B (full DMA rate).
    def xcols(c0, c1):
        return (xT_sb[:, :, c0:c1],
                xT_d[:, c0:c1].rearrange("(kt p) s -> p kt s", kt=KT))

    nc.sync.dma_start(
        out=wk_sb[:, :, 0:256],
        in_=wk_d[:, 0:256].rearrange("(kt p) c -> p kt c", kt=KT))
    nc.scalar.dma_start(
        out=wq_sb[:, :, 0:256],
        in_=wq_d[:, 0:256].rearrange("(kt p) c -> p kt c", kt=KT))
    o, i = xcols(0, 512)
    nc.sync.dma_start(out=o, in_=i)
    o, i = xcols(512, 1024)
    nc.scalar.dma_start(out=o, in_=i)
    nc.sync.dma_start(out=sin_sb[:, 0:1024], in_=sin_d[:, 0:1024])
    nc.scalar.dma_start(out=cos_sb[:, 0:1024], in_=cos_d[:, 0:1024])
    nc.sync.dma_start(
        out=wv_sb[:, 0:4, :],
        in_=wv_d[0:512, :].rearrange("(kt p) c -> p kt c", kt=4))
    nc.scalar.dma_start(
        out=wv_sb[:, 4:8, :],
        in_=wv_d[512:1024, :].rearrange("(kt p) c -> p kt c", kt=4))
    nc.scalar.dma_start(out=msk_sb, in_=msk_d)
    o, i = xcols(1024, 1536)
    nc.sync.dma_start(out=o, in_=i)
    nc.scalar.dma_start(out=cos_sb[:, 1024:S], in_=cos_d[:, 1024:S])
    nc.sync.dma_start(out=sin_sb[:, 1024:S], in_=sin_d[:, 1024:S])
    o, i = xcols(1536, 2048)
    nc.scalar.dma_start(out=o, in_=i)
    nc.sync.dma_start(
        out=wk_sb[:, :, 256:512],
        in_=wk_d[:, 256:512].rearrange("(kt p) c -> p kt c", kt=KT))
    nc.scalar.dma_start(
        out=wq_sb[:, :, 256:512],
        in_=wq_d[:, 256:512].rearrange("(kt p) c -> p kt c", kt=KT))
    nc.sync.dma_start(
        out=wp_sb[:], in_=wp_d.rearrange("(t p) c -> p t c", t=4))

    def accum512(dst_view, lhsT_of_kt, rhs_of_kt, name, src_rearrange=None):
        """8-step k-accumulation into a [128, 512] aux psum, evac'd to dst."""
        ps = auxp.tile([128, 512], F32, tag="aux", name=name + R)
        for kt in range(KT):
            nc.tensor.matmul(
                ps[:], lhsT_of_kt(kt), rhs_of_kt(kt),
                start=(kt == 0), stop=(kt == KT - 1),
            )
        src = ps[:] if src_rearrange is None else ps[:].rearrange(*src_rearrange, d=HD)
        nc.vector.tensor_copy(dst_view, src)

    def emit_v(sc):
        accum512(
            v_sb[:, sc, :, 0:HD],
            lambda kt, sc=sc: xT_sb[:, kt, sc * 128:(sc + 1) * 128],
            lambda kt: wv_sb[:, kt, :],
            name=f"vps{sc}",
            src_rearrange=("p (h d) -> p h d",),
        )

    rope_raw = {}

    def emit_kq_quarter(t, which, qc):
        w_sb = wk_sb if which == 0 else wq_sb
        if qc == 0:
            rope_raw[(t, which)] = ropep.tile(
                [128, S], FP16, tag="raw", bufs=2, name=f"raw{t}_{which}" + R)
        raw = rope_raw[(t, which)]
        accum512(
            raw[:, qc * 512:(qc + 1) * 512],
            lambda kt, t=t, w_sb=w_sb: w_sb[:, kt, t * 128:(t + 1) * 128],
            lambda kt, qc=qc: xT_sb[:, kt, qc * 512:(qc + 1) * 512],
            name=f"kq{t}_{which}_{qc}",
        )

    def emit_rope_q(t, which, qc):
        raw = rope_raw[(t, which)]
        dst = kT if which == 0 else qT
        cs = slice(qc * 512, qc * 512 + 512)
        sw = ropep.tile([128, 512], FP16, tag="sw", bufs=2,
                        name=f"sw{t}_{which}_{qc}" + R)
        nc.vector.stream_shuffle(sw[:], raw[:, cs], SWAP_MASK)
        nc.vector.tensor_mul(raw[:, cs], raw[:, cs], cos_sb[:, cs])
        nc.vector.tensor_mul(sw[:], sw[:], sin_sb[:, cs])
        nc.vector.tensor_add(dst[t][:, cs], raw[:, cs], sw[:])
        if qc == 3:
            rope_raw.pop((t, which))

    def emit_proj(ts, dst_d, sc, tail=False, ev="v", osb_view=None,
                  do_dma=True):
        """Projection of seq chunk sc, accumulating tiles `ts` in PSUM
        before one evacuation + one per-chunk DMA. ev: "v" = both
        evacuations on DVE, "sv" = ScalarE + DVE (for segments where DVE
        is busy with RoPE). osb_view/do_dma let the tail pair chunks into
        one larger DMA."""
        if osb_view is not None:
            osb = osb_view
        else:
            osb = outp.tile([128, DM], FP16, tag="osb",
                            name=f"osb{'_'.join(map(str, ts))}_{sc}" + R)
        for nn in range(2):
            # tail pps alternate between the aux and (idle) scores pools:
            # 4 psum slots in flight so evacuations never stall the PE
            pool = (spsp if sc % 2 else auxp) if tail else auxp
            pp = pool.tile([128, 512], F32,
                           tag="s" if pool is spsp else "aux",
                           name=f"pp{ts[0]}_{sc}_{nn}" + R)
            for j, t in enumerate(ts):
                nc.tensor.matmul(
                    pp[:],
                    yn[t][:, sc * 128:(sc + 1) * 128],
                    wp_sb[:, t, nn * 512:(nn + 1) * 512],
                    start=(j == 0),
                    stop=(j == len(ts) - 1),
                )
            dst = osb[:, nn * 512:(nn + 1) * 512]
            # PSUM evacuations: GPSIMD cannot touch PSUM on TRN2, so they
            # go to DVE, with ScalarE taking a share when asked (and most
            # of the tail, where DVE runs the normalize slivers)
            if tail:
                use_scalar = nn == 0 or sc % 2 == 0
            else:
                use_scalar = ev == "sv" and nn == 0
            if use_scalar:
                nc.scalar.copy(dst, pp[:])
            else:
                nc.vector.tensor_copy(dst, pp[:])
            if tail and sc == SC - 1:
                # DMA each half as soon as its evacuation lands
                nc.sync.dma_start(
                    out=dst_d[sc * 128:(sc + 1) * 128,
                              nn * 512:(nn + 1) * 512],
                    in_=dst)
        if do_dma and not (tail and sc == SC - 1):
            nc.sync.dma_start(
                out=dst_d[sc * 128:(sc + 1) * 128, :], in_=osb[:])

    # ---- prologue: rope'd K/Q quarters 0-1 gate the first scores.
    # warm2() bridges keep the PE p-state up across DMA waits (rep 0 only -
    # later reps overlap the previous rep's tail).
    emit_kq_quarter(0, 0, 0)
    emit_kq_quarter(0, 1, 0)
    emit_rope_q(0, 0, 0)
    emit_rope_q(0, 1, 0)
    if warm2:
        warm2(9)
    emit_kq_quarter(0, 1, 1)
    emit_rope_q(0, 1, 1)

    sps_tiles = {}
    segs = [(t, par, qh) for t in range(3) for par in (0, 1)
            for qh in (0, 1)]
    # tile 3 runs both qh0 segments first so yn[3] qh0 completes a
    # segment earlier and the p3 projections spread over two segments
    segs += [(3, 0, 0), (3, 1, 0), (3, 0, 1), (3, 1, 1)]

    def emit_scores(si, kc):
        t, par, qh = segs[si]
        rows = slice(64 * par, 64 * par + 64)
        sps = spsp.tile([128, 1024], F32, tag="s",
                        name=f"s{t}_{par}_{qh}_{kc}" + R)
        for qg in range(2):
            q0 = qh * 1024 + qg * 512
            nc.tensor.matmul(
                sps[:, qg * 512:(qg + 1) * 512],
                kT[t][rows, kc * 128:(kc + 1) * 128],
                qT[t][rows, q0:q0 + 512],
                start=True,
                stop=True,
            )
        sps_tiles[(si, kc)] = sps

    emit_v(0)
    emit_v(1)
    emit_kq_quarter(0, 0, 1)
    emit_rope_q(0, 0, 1)
    if warm2:
        warm2(4)
    emit_scores(0, 0)
    emit_scores(0, 1)
    emit_v(2)

    # ---- static per-slot filler plan ----
    plan = {}

    def put(si, kc, fn):
        plan.setdefault((si, kc), []).append(fn)

    def kq(t, which, qc):
        return lambda: emit_kq_quarter(t, which, qc)

    def rope(t, which, qc):
        return lambda: emit_rope_q(t, which, qc)

    def p01(sc, ev="v"):
        return lambda: emit_proj((0, 1), out01_d, sc, ev=ev)

    def p2(sc, ev="v"):
        return lambda: emit_proj((2,), out2_d, sc, ev=ev)

    def p3(sc, ev="v"):
        return lambda: emit_proj((3,), out3_d, sc, ev=ev)

    def spread(si, items):
        n = len(items)
        for j, fn in enumerate(items):
            put(si, (j * SC + SC // 2) // max(n, 1), fn)

    # seg 0: V chunks just-in-time (one slot early) + tile-0 K/Q quarters
    # 2-3 where xT has landed.
    for sc in range(3, SC):
        put(0, sc - 2, lambda sc=sc: emit_v(sc))
    put(0, 3, kq(0, 0, 2)); put(0, 4, rope(0, 0, 2))
    put(0, 5, kq(0, 0, 3)); put(0, 6, rope(0, 0, 3))
    put(0, 7, kq(0, 1, 2)); put(0, 8, rope(0, 1, 2))
    put(0, 9, kq(0, 1, 3)); put(0, 10, rope(0, 1, 3))

    # middle segments: K/Q of the next tiles + projections, balanced to
    # ~5-7us of filler PE per segment. Projection windows respect when the
    # yn halves they read are written (a segment's normalize is at its END).
    spread(1, [kq(1, 0, 0), rope(1, 0, 0), kq(1, 0, 1), rope(1, 0, 1),
               kq(1, 0, 2), rope(1, 0, 2)])
    spread(2, [kq(1, 0, 3), rope(1, 0, 3), kq(1, 1, 0), rope(1, 1, 0),
               kq(1, 1, 1), rope(1, 1, 1)])
    spread(3, [kq(1, 1, 2), rope(1, 1, 2), kq(1, 1, 3), rope(1, 1, 3),
               kq(2, 0, 0), rope(2, 0, 0)])
    spread(4, [kq(2, 0, 1), rope(2, 0, 1), kq(2, 0, 2), rope(2, 0, 2),
               kq(2, 0, 3), rope(2, 0, 3)])
    spread(5, [kq(2, 1, 0), rope(2, 1, 0), kq(2, 1, 1), rope(2, 1, 1),
               kq(2, 1, 2), rope(2, 1, 2)])
    spread(6, [kq(2, 1, 3), rope(2, 1, 3), kq(3, 0, 0), rope(3, 0, 0),
               kq(3, 0, 1), rope(3, 0, 1)])
    spread(7, [p01(0), p01(1), p01(2), p01(3)])
    spread(8, [p01(4), p01(5), p01(6), p01(7)])
    spread(9, [kq(3, 0, 2), rope(3, 0, 2), kq(3, 0, 3), rope(3, 0, 3),
               p01(8), p01(9)])
    spread(10, [kq(3, 1, 0), rope(3, 1, 0), kq(3, 1, 1), rope(3, 1, 1),
                p01(10), p01(11)])
    spread(11, [kq(3, 1, 2), rope(3, 1, 2), kq(3, 1, 3), rope(3, 1, 3),
                p2(0), p2(1), p2(2), p2(3)])
    spread(12, [p01(12), p01(13), p2(4), p2(5), p2(6), p2(7)])
    spread(13, [p01(14), p01(15), p2(8), p2(9), p2(10), p2(11)])
    spread(14, [p2(12), p2(13), p2(14), p2(15), p3(0), p3(1)])
    spread(15, [p3(2), p3(3), p3(4), p3(5), p3(6), p3(7)])

    # ---- global chunk pipeline over all 16 segments ----
    for si, (t, par, qh) in enumerate(segs):
        h = 2 * t + par
        rows = slice(64 * par, 64 * par + 64)
        pv = pvpsp.tile([128, 1024], F32, tag="pv", name=f"pv{h}_{qh}" + R)
        for kc in range(SC):
            sps = sps_tiles.pop((si, kc))
            pt = ptp.tile([128, 1024], FP16, tag="pt",
                          name=f"pt{h}_{qh}_{kc}" + R)
            nc.scalar.activation(pt[:], sps[:], Exp, scale=SCALE)
            c0 = kc * 128 - qh * 1024 if kc // 8 == qh else None
            if c0 is not None:
                nc.gpsimd.tensor_mul(
                    pt[:, c0:c0 + 128], pt[:, c0:c0 + 128], msk_sb[:])
            for qg in range(2):
                cols = [(qg * 512, (qg + 1) * 512)]
                if (c0 is not None and qg * 512 <= c0 < (qg + 1) * 512
                        and kc != 0):
                    # split around the masked diag block so only those 128
                    # columns wait for the Pool mask; the rest start right
                    # after the exp. Never at kc==0: two start=True matmuls
                    # into one psum bank would reset each other's piece.
                    cols = [(a, b) for a, b in
                            ((qg * 512, c0), (c0 + 128, (qg + 1) * 512))
                            if a < b] + [(c0, c0 + 128)]
                for a, b in cols:
                    nc.tensor.matmul(
                        pv[:, a:b],
                        v_sb[:, kc, h, :],
                        pt[:, a:b],
                        start=(kc == 0),
                        stop=(kc == SC - 1),
                    )
            # scores prefetch runs 2 chunks ahead, crossing segment
            # boundaries so the PE stream never restarts cold; emitted
            # before the fillers so the next exp gets extra slack.
            if kc + 2 < SC:
                emit_scores(si, kc + 2)
            elif si + 1 < len(segs):
                emit_scores(si + 1, kc + 2 - SC)
            for fn in plan.get((si, kc), ()):
                fn()
        # normalize: rows 64..127 of pv hold the denominators replicated,
        # courtesy of the ones-columns in v_sb.
        q0 = qh * 1024
        if si == 15:
            # tail: normalize per 128-col seq chunk, then project it
            # immediately (t3 partial -> out3 rows 1024:2047)
            ya = yap.tile([128, 1024], FP16, tag="ya", name="yat" + R)
            pair = None
            for sc in range(8, SC):
                gs = slice((sc - 8) * 128, (sc - 7) * 128)
                nc.vector.tensor_copy(ya[:, gs], pv[:, gs])
                rz = rzbp.tile([64, 128], F32, tag="rz", name=f"rzt{sc}" + R)
                nc.vector.reciprocal(rz[:], ya[64:128, gs])
                nc.vector.tensor_mul(
                    yn[t][rows, q0 + (sc - 8) * 128:q0 + (sc - 7) * 128],
                    ya[0:64, gs], rz[:])
                if sc < 14:
                    # pair chunks into one [256, DM] DMA - fewer trips
                    # through the serial HWDGE at the end of the kernel
                    if sc % 2 == 0:
                        pair = outp.tile([128, 2, DM], FP16, tag="osb",
                                         name=f"osbp{sc}" + R)
                    emit_proj((3,), out3_d, sc, tail=True,
                              osb_view=pair[:, sc % 2, :], do_dma=False)
                    if sc % 2 == 1:
                        nc.sync.dma_start(
                            out=out3_d[(sc - 1) * 128:(sc + 1) * 128, :]
                            .rearrange("(j p) c -> p j c", j=2),
                            in_=pair[:])
                else:
                    emit_proj((3,), out3_d, sc, tail=True)
        else:
            ya = yap.tile([128, 1024], FP16, tag="ya", name=f"ya{h}_{qh}" + R)
            nc.vector.tensor_copy(ya[:], pv[:])
            rz = rzbp.tile([64, 1024], F32, tag="rz", name=f"rz{h}_{qh}" + R)
            nc.vector.reciprocal(rz[:], ya[64:128, :])
            nc.vector.tensor_mul(
                yn[t][rows, q0:q0 + 1024], ya[0:64, :], rz[:])


def _host_tables():
    theta = 1.0 / (10000.0 ** (np.arange(0, HD, 2, dtype=np.float32) / HD))
    ang = np.arange(S, dtype=np.float32)[:, None] * theta[None, :]  # [S, 32]
    cos = np.repeat(np.cos(ang).T, 2, axis=0)  # [64, S]
    sin_ = np.empty((HD, S), np.float32)
    sin_[0::2] = -np.sin(ang).T
    sin_[1::2] = np.sin(ang).T
    cosb = np.concatenate([cos, cos], axis=0).astype(np.float16)  # [128, S]
    sinb = np.concatenate([sin_, sin_], axis=0).astype(np.float16)
    dmask = (1.0 - np.eye(128, dtype=np.float32)).astype(np.float16)
    return cosb, sinb, dmask


def _in_maps(x, Wqkv, Wproj):
    cosb, sinb, dmask = _host_tables()
    maps = []
    for core in range(N_CORES):
        b, hh = divmod(core, 2)
        c0 = hh * 512
        maps.append(
            {
                "xT": np.ascontiguousarray(x[b].T).astype(np.float16),
                "wq": np.ascontiguousarray(Wqkv[:, c0:c0 + 512]).astype(np.float16),
                "wk": np.ascontiguousarray(Wqkv[:, DM + c0:DM + c0 + 512]).astype(np.float16),
                "wv": np.ascontiguousarray(Wqkv[:, 2 * DM + c0:2 * DM + c0 + 512]).astype(np.float16),
                "wp": np.ascontiguousarray(Wproj[c0:c0 + 512, :]).astype(np.float16),
                "cosb": cosb,
                "sinb": sinb,
                "dmask": dmask,
            }
        )
    return maps


def kernel(x, Wqkv, Wproj):
    if "nc" not in _CACHE:
        _CACHE["nc"] = _build_nc()
    nc = _CACHE["nc"]

    x = np.asarray(x)
    Wqkv = np.asarray(Wqkv)
    Wproj = np.asarray(Wproj)

    res = run_bass_kernel_spmd(nc, _in_maps(x, Wqkv, Wproj), core_ids=list(range(N_CORES)))
    out = np.empty((B, S, DM), np.float32)
    for b in range(B):
        acc = np.zeros((S, DM), np.float32)
        for core in (2 * b, 2 * b + 1):
            r = res.results[core]
            acc += r["out01"]
            acc += r["out2"]
            acc += r["out3"]
        out[b] = acc
    return out
